# revision 9
# baseline (speedup 1.0000x reference)
"""Trainium2 Bass kernel for nn_CrossProduct (factorization-machine cross term).

out_b = 0.5 * [ sum_k (x_b @ v_k)^2  -  sum_i w_i x_bi^2 ],  w_i = sum_k v_ik^2

Host-side rescaling removes all per-feature weights from the device:
  x~  = e3m4(4 * x * sqrt(w/2))     (shipped fp8e3m4, feature-on-partition)
  v~  = fp16(v / sqrt(w))           (replicated fp16)
  psA[k,b] = sum_i x~_bi v~_ik = 4 * (x v_k)/sqrt(2)
  sq = (psA/4)^2 = (xv)^2/2  (fp16)
  s2[p,b] = sum_c x~^2  accumulated over chunks; term2 = s2/16
  out_b = sum_k sq[k,b] - s2_b/16   via ONE 128-deep ones-matmul over
          stacked [sq (64p, +1) ; s2h (64p, -1/16)].

Device program per core (2048 batch rows, 8 contraction chunks of 128):
  - X dram layout [128, 8, 2048] e3m4: per-partition contiguous 16KB;
    chunk-pair DMAs have 4KB descriptors (queue packet pacing ~21ns/pkt
    caps a queue at ~pktsize/21ns; 4KB keeps 2 queues above the 16-engine
    ~350GB/s aggregate).  Pairs (0,1),(4,5) on sync HWDGE; (2,3),(6,7) on
    gpsimd SWDGE.
  - PE: warm-up matmuls (junk into psO bank) to trigger p-state ramp-up,
    then per chunk one fused 2048-col matmul into psA[64,2048] (fp16
    weights x e3m4 moving), then sq reduce + s2h reduce in one final
    128-deep pass into psO row 64.
  - x~^2 squares + s2 accumulation tree split across ACT/DVE/Pool.
"""

import math
from contextlib import ExitStack

import ml_dtypes
import numpy as np

import concourse.bass as bass
import concourse.bacc as bacc
import concourse.mybir as mybir
import concourse.tile as tile
from concourse.bass_utils import run_bass_kernel_spmd

F16 = mybir.dt.float16
F32 = mybir.dt.float32
F8E3 = mybir.dt.float8e3

N_CORES = 8
B, XD, KD = 16384, 1024, 64
BS = B // N_CORES   # 2048 batch rows per core
C = XD // 128       # 8 contraction chunks of 128
ALPHA = 4.0         # x pre-scale (exact pow2); term2 weight = 1/ALPHA^2

SQ = mybir.ActivationFunctionType.Square


def _body(ctx, tc, OUT, X, VW):
    nc = tc.nc
    const = ctx.enter_context(tc.tile_pool(name="const", bufs=1))
    xpool = ctx.enter_context(tc.tile_pool(name="xp", bufs=1))
    x2pool = ctx.enter_context(tc.tile_pool(name="x2p", bufs=1))
    spool = ctx.enter_context(tc.tile_pool(name="sp", bufs=1))
    sqpool = ctx.enter_context(tc.tile_pool(name="sqp", bufs=1))
    opool = ctx.enter_context(tc.tile_pool(name="op", bufs=1))
    psa = ctx.enter_context(tc.tile_pool(name="psA", bufs=1, space="PSUM"))
    pso = ctx.enter_context(tc.tile_pool(name="psO", bufs=1, space="PSUM"))

    # vw cols: [c*64:(c+1)*64] = v~_c ; col 512 = +1 ; col 513 = -1/16.
    vw = const.tile([128, C * KD + 2], F16)
    nc.scalar.dma_start(vw[:], VW)

    pa = psa.tile([64, BS], F32)
    po = pso.tile([65, BS], F32)

    # ---- PE warm-up: junk matmuls to start the p-state ramp while the
    # first X chunks are still in flight. Output goes to psO bank cols
    # 0:512 which the real final matmul later resets (start=True).
    ones = vw[:, C * KD : C * KD + 1]
    for _ in range(7):
        nc.tensor.matmul(
            po[64:65, 0:512], ones, vw[:, 0:512],
            start=True, stop=True, tile_position=(0, 64),
        )

    # ---- X input: chunk-pair DMAs, 4KB/partition descriptors.
    xt = xpool.tile([128, C, BS], F8E3)
    nc.sync.dma_start(xt[:, 0:2], X[:, 0:2])
    nc.gpsimd.dma_start(xt[:, 2:4], X[:, 2:4])
    nc.sync.dma_start(xt[:, 4:6], X[:, 4:6])
    nc.gpsimd.dma_start(xt[:, 6:8], X[:, 6:8])

    # ---- pa matmuls: per chunk, fp16 weights (stationary) x e3m4 moving.
    # PSUM bank limit -> 512-column splits.
    def pa_mm(c):
        for q in range(4):
            nc.tensor.matmul(
                pa[:, q * 512 : (q + 1) * 512],
                vw[:, c * KD : (c + 1) * KD],
                xt[:, c, q * 512 : (q + 1) * 512],
                start=(c == 0),
                stop=(c == C - 1),
                tile_position=(0, 0),
            )

    for c in range(C):
        pa_mm(c)

    # ---- squares x~^2 -> fp16, split across engines by (chunk, half).
    # ACT is fastest per fp8 element (0.83ns), DVE 1.04ns, Pool ~2ns.
    x2 = x2pool.tile([128, C, BS], F16)
    H = BS // 2

    def sq_act(c, h):
        nc.scalar.activation(x2[:, c, h * H : (h + 1) * H],
                             xt[:, c, h * H : (h + 1) * H], SQ)

    def sq_dve(c, h):
        nc.vector.tensor_mul(x2[:, c, h * H : (h + 1) * H],
                             xt[:, c, h * H : (h + 1) * H],
                             xt[:, c, h * H : (h + 1) * H])

    def sq_pool(c, h):
        nc.gpsimd.tensor_mul(x2[:, c, h * H : (h + 1) * H],
                             xt[:, c, h * H : (h + 1) * H],
                             xt[:, c, h * H : (h + 1) * H])

    # chunks 0-3 land first (~2.9us), 4-7 by ~5.7us.
    for c, h in [(0, 0), (0, 1), (1, 0), (1, 1), (4, 0), (4, 1), (5, 0)]:
        sq_act(c, h)
    for c, h in [(2, 0), (2, 1), (3, 0), (5, 1), (6, 0), (6, 1)]:
        sq_dve(c, h)
    for c, h in [(3, 1), (7, 0), (7, 1)]:
        sq_pool(c, h)

    # ---- s2 accumulation tree (fp16, DVE 2x mode).
    p01 = spool.tile([128, BS], F16)
    p23 = spool.tile([128, BS], F16)
    p45 = spool.tile([128, BS], F16)
    p67 = spool.tile([128, BS], F16)
    s2a = spool.tile([128, BS], F16)
    s2b = spool.tile([128, BS], F16)
    s2 = spool.tile([128, BS], F16)
    nc.vector.tensor_add(p01, x2[:, 0], x2[:, 1])
    nc.vector.tensor_add(p23, x2[:, 2], x2[:, 3])
    nc.vector.tensor_add(p45, x2[:, 4], x2[:, 5])
    nc.gpsimd.tensor_add(p67, x2[:, 6], x2[:, 7])
    nc.vector.tensor_add(s2a, p01, p23)
    nc.vector.tensor_add(s2b, p45, p67)
    nc.vector.tensor_add(s2, s2a, s2b)

    # ---- sq = (psA/4)^2 fp16 (ACT reads PSUM directly).
    sqs = sqpool.tile([64, BS], F16)
    nc.scalar.activation(sqs[:, 0:H], pa[:, 0:H], SQ, scale=1.0 / ALPHA)
    nc.scalar.activation(sqs[:, H:BS], pa[:, H:BS], SQ, scale=1.0 / ALPHA)

    # ---- final: out row = ones64^T sq - (1/16) ones128^T s2, two passes
    # accumulated into psO row 64.
    ones64 = vw[0:64, C * KD : C * KD + 1]
    wneg = vw[:, C * KD + 1 : C * KD + 2]
    for q in range(4):
        nc.tensor.matmul(
            po[64:65, q * 512 : (q + 1) * 512],
            ones64,
            sqs[:, q * 512 : (q + 1) * 512],
            start=True,
            stop=False,
            tile_position=(0, 64),
        )
    for q in range(4):
        nc.tensor.matmul(
            po[64:65, q * 512 : (q + 1) * 512],
            wneg,
            s2[:, q * 512 : (q + 1) * 512],
            start=False,
            stop=True,
            tile_position=(0, 64),
        )

    outs = opool.tile([65, BS], F32)
    nc.scalar.copy(outs[64:65, 0:H], po[64:65, 0:H])
    nc.vector.tensor_scalar_mul(outs[64:65, H:BS], po[64:65, H:BS], 1.0)
    nc.sync.dma_start(OUT, outs[64:65, :])


_NC_CACHE = None


def build_nc():
    global _NC_CACHE
    if _NC_CACHE is not None:
        return _NC_CACHE
    nc = bacc.Bacc("TRN2", target_bir_lowering=False, debug=False)
    X = nc.dram_tensor("X", [128, C, BS], F8E3, kind="ExternalInput").ap()
    VW = nc.dram_tensor("VW", [128, C * KD + 2], F16, kind="ExternalInput").ap()
    OUT = nc.dram_tensor("OUT", [1, BS], F32, kind="ExternalOutput").ap()
    with tile.TileContext(nc) as tc:
        with ExitStack() as ctx:
            _body(ctx, tc, OUT, X, VW)
    nc.compile()
    _NC_CACHE = nc
    return nc


def make_in_maps(x, vparam):
    x = np.ascontiguousarray(x, dtype=np.float32)
    v = np.ascontiguousarray(vparam, dtype=np.float32)

    w = (v.astype(np.float64) ** 2).sum(axis=1)          # (1024,)
    w = np.maximum(w, 1e-12)
    s = np.sqrt(w / 2.0)
    vn = (v / np.sqrt(w)[:, None]).astype(np.float32)    # (1024, 64)

    VWh = np.empty((128, C * KD + 2), dtype=np.float16)
    # VW[p, c*64+k] = vn[c*128+p, k]
    VWh[:, 0 : C * KD] = (
        vn.reshape(C, 128, KD).transpose(1, 0, 2).reshape(128, C * KD)
    )
    VWh[:, C * KD] = 1.0
    VWh[:, C * KD + 1] = -1.0 / (ALPHA * ALPHA)

    xs_all = (ALPHA * x * s[None, :]).astype(ml_dtypes.float8_e3m4)  # (B, 1024)

    in_maps = []
    for i in range(N_CORES):
        xs = xs_all[i * BS : (i + 1) * BS]               # (2048, 1024)
        # X[p, c, b] = xs.T[c*128+p, b]
        A = xs.T.reshape(C, 128, BS).transpose(1, 0, 2)
        in_maps.append({"X": np.ascontiguousarray(A), "VW": VWh})
    return in_maps


LAST_RESULTS = None  # stashed BassKernelResults (for test harness profiling)
TRACE = False


def kernel(x, vparam):
    global LAST_RESULTS
    nc = build_nc()
    in_maps = make_in_maps(x, vparam)
    res = run_bass_kernel_spmd(nc, in_maps, list(range(N_CORES)), trace=TRACE)
    LAST_RESULTS = res
    out = np.concatenate(
        [
            res.results[i]["OUT"].astype(np.float32).reshape(BS, 1)
            for i in range(N_CORES)
        ],
        axis=0,
    )
    return out.astype(np.float32)


# revision 13
# speedup vs baseline: 1.2541x; 1.2541x over previous
"""Trainium2 Bass kernel for nn_CrossProduct (factorization-machine cross term).

out_b = 0.5 * [ sum_k (x_b @ v_k)^2  -  sum_i w_i x_bi^2 ],  w_i = sum_k v_ik^2

Host-side rescaling removes all per-feature weights from the device:
  x~  = e3m4(4 * x * sqrt(w/2))     (shipped fp8e3m4, feature-on-partition)
  v~  = fp16(v / sqrt(w))           (bit-cast into the head of the X tensor)
  psA[k,b] = sum_i x~_bi v~_ik = 4 * (x v_k)/sqrt(2)     (mixed-dtype matmul)
  sq = (psA/4)^2 = (xv)^2/2  (fp16)
  x2 = x~^2 in e4m3, pair-interleaved -> term2 reduced on the PE via ONE
  DoubleRow ones(-1/16) matmul chain (4 passes of 256-deep contraction).
  out_b = sum_k sq[k,b] - (1/16) sum_i x2 accumulated in psO row 64.

Schedule highlights:
  - Single dram tensor XR [128, 17412] e3m4: bytes [0:1028) = vw (fp16
    bit-cast), then 8 chunks of 2048.  DMAs: sync (vw+c01, then c67),
    gpsimd SWDGE (c23, then c45); 4-8KB descriptors keep both queues at
    ~the 16-DMA-engine aggregate (~350GB/s).
  - PE warm-up matmuls on a memset tile (no DMA dependency) raise the
    p-state clock during the framework preamble.
  - squares x~ -> x2 split across ACT/DVE/Pool by (chunk, half), ordered
    by DMA arrival.
"""

import math
from contextlib import ExitStack

import ml_dtypes
import numpy as np

import concourse.bass as bass
import concourse.bacc as bacc
import concourse.mybir as mybir
import concourse.tile as tile
from concourse.bass_utils import run_bass_kernel_spmd

F16 = mybir.dt.float16
F32 = mybir.dt.float32
F8E3 = mybir.dt.float8e3
F8E4 = mybir.dt.float8e4

N_CORES = 8
B, XD, KD = 16384, 1024, 64
BS = B // N_CORES   # 2048 batch rows per core
C = XD // 128       # 8 contraction chunks of 128
ALPHA = 4.0         # x pre-scale (exact pow2); term2 weight = 1/ALPHA^2
CBIAS = 1.17        # x2 pre-cast scale nulling the e4m3 RTN square bias
VWB = 1028          # vw bytes per partition at the head of XR
XRW = VWB + C * BS  # 17412 bytes per partition

SQ = mybir.ActivationFunctionType.Square
DR = mybir.MatmulPerfMode.DoubleRow


def _body(ctx, tc, OUT, XR):
    nc = tc.nc
    const = ctx.enter_context(tc.tile_pool(name="const", bufs=1))
    xpool = ctx.enter_context(tc.tile_pool(name="xp", bufs=1))
    x2pool = ctx.enter_context(tc.tile_pool(name="x2p", bufs=1))
    sqpool = ctx.enter_context(tc.tile_pool(name="sqp", bufs=1))
    opool = ctx.enter_context(tc.tile_pool(name="op", bufs=1))
    psa = ctx.enter_context(tc.tile_pool(name="psA", bufs=1, space="PSUM"))
    pso = ctx.enter_context(tc.tile_pool(name="psO", bufs=1, space="PSUM"))

    pa = psa.tile([64, BS], F32)
    po = pso.tile([1, BS], F32)

    # ---- PE warm-up on a zeroed tile: no DMA dependency, runs during the
    # framework preamble so the PE clock is ramped when pa starts.
    warm = const.tile([128, 512], F16)
    nc.vector.memset(warm, 0.0)
    for _ in range(6):
        nc.tensor.matmul(
            po[0:1, 0:512], warm[:, 0:1], warm[:],
            start=True, stop=True, tile_position=(0, 0),
        )

    # ---- XR input: vw head + chunk pairs; sync gets (vw,c0,c1) and
    # (c6,c7), gpsimd SWDGE gets (c2,c3) and (c4,c5).
    xr = xpool.tile([128, XRW], F8E3)
    nc.sync.dma_start(xr[:, 0 : VWB + 2 * BS], XR[:, 0 : VWB + 2 * BS])
    nc.gpsimd.dma_start(
        xr[:, VWB + 2 * BS : VWB + 4 * BS], XR[:, VWB + 2 * BS : VWB + 4 * BS]
    )
    nc.sync.dma_start(
        xr[:, VWB + 6 * BS : VWB + 8 * BS], XR[:, VWB + 6 * BS : VWB + 8 * BS]
    )
    nc.gpsimd.dma_start(
        xr[:, VWB + 4 * BS : VWB + 6 * BS], XR[:, VWB + 4 * BS : VWB + 6 * BS]
    )

    # vw view: [128, 514] fp16 = [c*64+k -> v~ ; col 512 = +1 ; col 513 = -1/16]
    vw = xr[:, 0:VWB].bitcast(F16)

    def xch(c):
        return xr[:, VWB + c * BS : VWB + (c + 1) * BS]

    # ---- pa matmuls: fp16 weights (stationary) x e3m4 moving, psum-bank
    # 512-col splits, chunk order = DMA arrival order.
    def pa_mm(c, first, last):
        for q in range(4):
            nc.tensor.matmul(
                pa[:, q * 512 : (q + 1) * 512],
                vw[:, c * KD : (c + 1) * KD],
                xch(c)[:, q * 512 : (q + 1) * 512],
                start=first,
                stop=last,
                tile_position=(0, 0),
            )

    order = [0, 1, 2, 3, 6, 7, 4, 5]
    for i, c in enumerate(order):
        pa_mm(c, i == 0, i == len(order) - 1)

    # ---- squares: chunks 0-5 -> e4m3 scaled by CBIAS (pair-interleaved for
    # DoubleRow); chunks 6,7 -> plain fp16 (exact square, weighted -c/16).
    x2 = x2pool.tile([128, 3, 2, BS], F8E4)
    x27 = x2pool.tile([128, 2, BS], F16)
    H = BS // 2

    def x2ap(c, h):
        if c >= 6:
            return x27[:, c - 6, h * H : (h + 1) * H]
        return x2[:, c // 2, c % 2, h * H : (h + 1) * H]

    # x2 = CBIAS * x~^2: the non-pow2 scale breaks the e3m4-grid alignment
    # so the e4m3 RTN cast is unbiased (CBIAS chosen to null the mean).
    CSQ = math.sqrt(CBIAS)

    def sq_act(c, h):
        nc.scalar.activation(x2ap(c, h), xch(c)[:, h * H : (h + 1) * H], SQ,
                             scale=(CSQ if c < 6 else 1.0))

    def sq_dve(c, h):
        if c < 6:
            nc.vector.scalar_tensor_tensor(
                x2ap(c, h), xch(c)[:, h * H : (h + 1) * H], CBIAS,
                xch(c)[:, h * H : (h + 1) * H],
                mybir.AluOpType.mult, mybir.AluOpType.mult)
        else:
            nc.vector.tensor_mul(x2ap(c, h), xch(c)[:, h * H : (h + 1) * H],
                                 xch(c)[:, h * H : (h + 1) * H])

    def sq_pool(c, h):
        assert c >= 6  # Pool has no scalar_tensor_tensor; fp16 chunks only
        nc.gpsimd.tensor_mul(x2ap(c, h), xch(c)[:, h * H : (h + 1) * H],
                             xch(c)[:, h * H : (h + 1) * H])

    # arrival: c0,c1 first (sync), then c2,c3 (swdge), c6,c7 (sync), c4,c5.
    for c, h in [(0, 0), (0, 1), (6, 0), (6, 1), (4, 0), (4, 1), (5, 0)]:
        sq_act(c, h)
    for c, h in [(1, 0), (1, 1), (2, 0), (2, 1), (3, 0), (3, 1), (5, 1)]:
        sq_dve(c, h)
    for c, h in [(7, 0), (7, 1)]:
        sq_pool(c, h)

    # ---- sq = (psA/4)^2 fp16 (ACT reads PSUM directly).
    sqs = sqpool.tile([64, BS], F16)
    nc.scalar.activation(sqs[:, 0:H], pa[:, 0:H], SQ, scale=1.0 / ALPHA)
    nc.scalar.activation(sqs[:, H:BS], pa[:, H:BS], SQ, scale=1.0 / ALPHA)

    # ---- finals into psO row 64, one accumulation group per 512-col bank:
    # 4 DoubleRow ones(-1/16) passes over x2 (256-deep each) + 1 fp16
    # ones(+1) pass over sq (64-deep).
    # DR weights need even, 16B-aligned k-tile stride -> [128, 2, 16] layout.
    wneg8 = const.tile([128, 2, 16], F8E4)
    nc.vector.memset(wneg8, -1.0 / (ALPHA * ALPHA))
    ones64 = vw[0:64, 512:513]
    wneg16 = vw[:, 513:514]
    for q in range(4):
        cols = slice(q * 512, (q + 1) * 512)
        for cc in range(3):
            nc.tensor.matmul(
                po[0:1, cols],
                wneg8[:, :, 0:1],
                x2[:, cc, :, cols],
                start=(cc == 0),
                stop=False,
                perf_mode=DR,
                tile_position=(0, 0),
            )
        for j in range(2):
            nc.tensor.matmul(
                po[0:1, cols],
                wneg16,
                x27[:, j, cols],
                start=False,
                stop=False,
                tile_position=(0, 0),
            )
        nc.tensor.matmul(
            po[0:1, cols],
            ones64,
            sqs[:, cols],
            start=False,
            stop=True,
            tile_position=(0, 0),
        )

    outs = opool.tile([1, BS], F32)
    nc.scalar.mul(outs[0:1, 0:H], po[0:1, 0:H], 1.0 / CBIAS)
    nc.vector.tensor_scalar_mul(outs[0:1, H:BS], po[0:1, H:BS], 1.0 / CBIAS)
    nc.sync.dma_start(OUT, outs[0:1, :])


_NC_CACHE = None


def build_nc():
    global _NC_CACHE
    if _NC_CACHE is not None:
        return _NC_CACHE
    nc = bacc.Bacc("TRN2", target_bir_lowering=False, debug=False)
    XR = nc.dram_tensor("XR", [128, XRW], F8E3, kind="ExternalInput").ap()
    OUT = nc.dram_tensor("OUT", [1, BS], F32, kind="ExternalOutput").ap()
    with tile.TileContext(nc) as tc:
        with ExitStack() as ctx:
            _body(ctx, tc, OUT, XR)
    nc.compile()
    _NC_CACHE = nc
    return nc


def make_in_maps(x, vparam):
    x = np.ascontiguousarray(x, dtype=np.float32)
    v = np.ascontiguousarray(vparam, dtype=np.float32)

    w = (v.astype(np.float64) ** 2).sum(axis=1)          # (1024,)
    w = np.maximum(w, 1e-12)
    s = np.sqrt(w / 2.0)
    vn = (v / np.sqrt(w)[:, None]).astype(np.float32)    # (1024, 64)

    VWh = np.empty((128, VWB // 2), dtype=np.float16)
    # VW[p, c*64+k] = vn[c*128+p, k]
    VWh[:, 0 : C * KD] = (
        vn.reshape(C, 128, KD).transpose(1, 0, 2).reshape(128, C * KD)
    )
    VWh[:, C * KD] = np.float16(1.17)        # CBIAS on the sq rows
    VWh[:, C * KD + 1] = np.float16(-1.17 / 16.0)  # -CBIAS/16 for fp16 chunks
    vw_bytes = VWh.view(np.uint8)                        # (128, 1028)

    xs_all = (ALPHA * x * s[None, :]).astype(ml_dtypes.float8_e3m4)  # (B, 1024)

    in_maps = []
    for i in range(N_CORES):
        xs = xs_all[i * BS : (i + 1) * BS]               # (2048, 1024)
        # chunk c bytes: [p, c*2048 + b] = xs.T[c*128+p, b]
        xb = np.ascontiguousarray(
            xs.T.reshape(C, 128, BS).transpose(1, 0, 2).reshape(128, C * BS)
        ).view(np.uint8)
        XRb = np.concatenate([vw_bytes, xb], axis=1)     # (128, 17412)
        in_maps.append({"XR": XRb.view(ml_dtypes.float8_e3m4)})
    return in_maps


LAST_RESULTS = None  # stashed BassKernelResults (for test harness profiling)
TRACE = False


def kernel(x, vparam):
    global LAST_RESULTS
    nc = build_nc()
    in_maps = make_in_maps(x, vparam)
    res = run_bass_kernel_spmd(nc, in_maps, list(range(N_CORES)), trace=TRACE)
    LAST_RESULTS = res
    out = np.concatenate(
        [
            res.results[i]["OUT"].astype(np.float32).reshape(BS, 1)
            for i in range(N_CORES)
        ],
        axis=0,
    )
    return out.astype(np.float32)


# revision 15
# speedup vs baseline: 1.3989x; 1.1154x over previous
"""Trainium2 Bass kernel for nn_CrossProduct (factorization-machine cross term).

out_b = 0.5 * [ sum_k (x_b @ v_k)^2  -  sum_i w_i x_bi^2 ],  w_i = sum_k v_ik^2

Host-side rescaling removes all per-feature weights from the device:
  x~  = e3m4(4 * x * sqrt(w/2))     (shipped fp8e3m4, feature-on-partition)
  v~  = fp16(v / sqrt(w))           (bit-cast into the head of the X tensor)
  psA[k,b] = sum_i x~_bi v~_ik = 4 * (x v_k)/sqrt(2)     (mixed-dtype matmul)
  sq = (psA/4)^2 fp16, weighted by fp16(CBIAS) in the final reduce
  x2 = e4m3(CBIAS * x~^2) pair-interleaved; reduced on the PE via four
  DoubleRow ones(-1/16) passes (256-deep contraction each).  CBIAS is a
  non-pow2 scale that breaks e3m4-grid alignment so the e4m3 RTN cast of
  squares is mean-unbiased (plain cast has +0.75% convexity bias).
  out_b = (sum_k c*sq - (1/16) sum x2) / c  accumulated in psO row 0.

Schedule highlights:
  - Single dram tensor XR [128, 17412] e3m4, chunk order
    [vw | c0 | c1 | c6 | c7 | c2 | c3 | c4 | c5]:  sync HWDGE sends
    [vw,c0] then [c1,c6,c7]; gpsimd SWDGE sends [c2,c3] then [c4,c5].
    3-6KB descriptors keep both queues near the 16-DMA-engine aggregate.
  - 12 PE warm-up matmuls on a memset tile (no DMA dependency) hold the
    p-state clock up until real work arrives.
  - squares split ACT/DVE by (chunk, half) in arrival order; Pool does
    no tensor ops (they contend with DVE for SBUF ports).
  - finals in weights-major order: one ldweights per weight set.
"""

import math
from contextlib import ExitStack

import ml_dtypes
import numpy as np

import concourse.bass as bass
import concourse.bacc as bacc
import concourse.mybir as mybir
import concourse.tile as tile
from concourse.bass_utils import run_bass_kernel_spmd

F16 = mybir.dt.float16
F32 = mybir.dt.float32
F8E3 = mybir.dt.float8e3
F8E4 = mybir.dt.float8e4

N_CORES = 8
B, XD, KD = 16384, 1024, 64
BS = B // N_CORES   # 2048 batch rows per core
C = XD // 128       # 8 contraction chunks of 128
ALPHA = 4.0         # x pre-scale (exact pow2); term2 weight = 1/ALPHA^2
CBIAS = 1.17        # x2 pre-cast scale nulling the e4m3 RTN square bias
VWB = 1028          # vw bytes per partition at the head of XR
XRW = VWB + C * BS  # 17412 bytes per partition

# chunk order inside XR (after vw), chosen so the sync queue can deliver
# c0 early and both queues balance.
XORD = [0, 1, 6, 7, 2, 3, 4, 5]
XPOS = {c: i for i, c in enumerate(XORD)}  # chunk -> slot

SQ = mybir.ActivationFunctionType.Square
DR = mybir.MatmulPerfMode.DoubleRow


def _body(ctx, tc, OUT, XR):
    nc = tc.nc
    const = ctx.enter_context(tc.tile_pool(name="const", bufs=1))
    xpool = ctx.enter_context(tc.tile_pool(name="xp", bufs=1))
    x2pool = ctx.enter_context(tc.tile_pool(name="x2p", bufs=1))
    sqpool = ctx.enter_context(tc.tile_pool(name="sqp", bufs=1))
    opool = ctx.enter_context(tc.tile_pool(name="op", bufs=1))
    psa = ctx.enter_context(tc.tile_pool(name="psA", bufs=1, space="PSUM"))
    pso = ctx.enter_context(tc.tile_pool(name="psO", bufs=1, space="PSUM"))

    pa = psa.tile([64, BS], F32)
    po = pso.tile([1, BS], F32)

    # ---- PE warm-up (alternating psO bank regions so they pipeline).
    warm = const.tile([128, 512], F16)
    nc.vector.memset(warm, 0.0)
    for i in range(12):
        cols = slice(512 * (i % 2), 512 * (i % 2) + 512)
        nc.tensor.matmul(
            po[0:1, cols], warm[:, 0:1], warm[:],
            start=True, stop=True, tile_position=(0, 0),
        )

    # ---- XR input.
    xr = xpool.tile([128, XRW], F8E3)

    def xsl(lo, hi):  # slot-range bounds (bytes, after vw head)
        return slice(VWB + lo * BS, VWB + hi * BS)

    nc.sync.dma_start(xr[:, 0 : VWB + BS], XR[:, 0 : VWB + BS])      # vw,c0
    nc.gpsimd.dma_start(xr[:, xsl(4, 6)], XR[:, xsl(4, 6)])          # c2,c3
    nc.sync.dma_start(xr[:, xsl(1, 4)], XR[:, xsl(1, 4)])            # c1,c6,c7
    nc.gpsimd.dma_start(xr[:, xsl(6, 8)], XR[:, xsl(6, 8)])          # c4,c5

    # vw view: [128, 514] fp16 = [c*64+k -> v~ ; col 512 = +CBIAS(fp16)]
    vw = xr[:, 0:VWB].bitcast(F16)

    def xch(c):
        i = XPOS[c]
        return xr[:, VWB + i * BS : VWB + (i + 1) * BS]

    # ---- pa matmuls in arrival order: fp16 weights x e3m4 moving.
    def pa_mm(c, first, last):
        for q in range(4):
            nc.tensor.matmul(
                pa[:, q * 512 : (q + 1) * 512],
                vw[:, c * KD : (c + 1) * KD],
                xch(c)[:, q * 512 : (q + 1) * 512],
                start=first,
                stop=last,
                tile_position=(0, 0),
            )

    order = [0, 2, 3, 1, 6, 7, 4, 5]
    for i, c in enumerate(order):
        pa_mm(c, i == 0, i == len(order) - 1)

    # ---- squares: x2 = e4m3(CBIAS * x~^2), pair-interleaved for DR.
    # x2[:, cc, j, :] holds chunk (2cc+j)^2.
    x2 = x2pool.tile([128, C // 2, 2, BS], F8E4)
    H = BS // 2
    CSQ = math.sqrt(CBIAS)

    def x2ap(c, h):
        return x2[:, c // 2, c % 2, h * H : (h + 1) * H]

    def sq_act(c, h):
        nc.scalar.activation(x2ap(c, h), xch(c)[:, h * H : (h + 1) * H], SQ,
                             scale=CSQ)

    def sq_dve(c, h):
        nc.vector.scalar_tensor_tensor(
            x2ap(c, h), xch(c)[:, h * H : (h + 1) * H], CBIAS,
            xch(c)[:, h * H : (h + 1) * H],
            mybir.AluOpType.mult, mybir.AluOpType.mult)

    # arrival: c0 ~11us; c2,c3 ~13.6; c1,c6,c7 ~15; c4,c5 ~16.3
    for c, h in [(0, 0), (0, 1), (2, 0), (2, 1), (6, 0), (6, 1), (4, 0), (4, 1)]:
        sq_act(c, h)
    for c, h in [(3, 0), (3, 1), (1, 0), (1, 1), (7, 0), (7, 1), (5, 0), (5, 1)]:
        sq_dve(c, h)

    # ---- sq = (psA/4)^2 fp16 on ACT (DVE cannot read two PSUM operands).
    sqs = sqpool.tile([64, BS], F16)
    nc.scalar.activation(sqs[:], pa[:], SQ, scale=1.0 / ALPHA)

    # ---- finals into psO row 0: weights-major order (one ldweights per
    # weight set): 4 DR ones(-1/16) passes over x2, then the +c sq pass.
    wneg8 = const.tile([128, 2, 16], F8E4)
    nc.vector.memset(wneg8, -1.0 / (ALPHA * ALPHA))
    onesc = vw[0:64, 512:513]
    for cc in range(4):
        for q in range(4):
            cols = slice(q * 512, (q + 1) * 512)
            nc.tensor.matmul(
                po[0:1, cols],
                wneg8[:, :, 0:1],
                x2[:, cc, :, cols],
                start=(cc == 0),
                stop=False,
                perf_mode=DR,
                tile_position=(0, 0),
            )
    for q in range(4):
        cols = slice(q * 512, (q + 1) * 512)
        nc.tensor.matmul(
            po[0:1, cols],
            onesc,
            sqs[:, cols],
            start=False,
            stop=True,
            tile_position=(0, 0),
        )

    outs = opool.tile([1, BS], F16)
    nc.scalar.mul(outs[0:1, 0:H], po[0:1, 0:H], 1.0 / CBIAS)
    nc.vector.tensor_scalar_mul(outs[0:1, H:BS], po[0:1, H:BS], 1.0 / CBIAS)
    nc.sync.dma_start(OUT, outs[0:1, :])


_NC_CACHE = None


def build_nc():
    global _NC_CACHE
    if _NC_CACHE is not None:
        return _NC_CACHE
    nc = bacc.Bacc("TRN2", target_bir_lowering=False, debug=False)
    XR = nc.dram_tensor("XR", [128, XRW], F8E3, kind="ExternalInput").ap()
    OUT = nc.dram_tensor("OUT", [1, BS], F16, kind="ExternalOutput").ap()
    with tile.TileContext(nc) as tc:
        with ExitStack() as ctx:
            _body(ctx, tc, OUT, XR)
    nc.compile()
    _NC_CACHE = nc
    return nc


def make_in_maps(x, vparam):
    x = np.ascontiguousarray(x, dtype=np.float32)
    v = np.ascontiguousarray(vparam, dtype=np.float32)

    w = (v.astype(np.float64) ** 2).sum(axis=1)          # (1024,)
    w = np.maximum(w, 1e-12)
    s = np.sqrt(w / 2.0)
    vn = (v / np.sqrt(w)[:, None]).astype(np.float32)    # (1024, 64)

    VWh = np.empty((128, VWB // 2), dtype=np.float16)
    # VW[p, c*64+k] = vn[c*128+p, k]
    VWh[:, 0 : C * KD] = (
        vn.reshape(C, 128, KD).transpose(1, 0, 2).reshape(128, C * KD)
    )
    VWh[:, C * KD] = np.float16(CBIAS)   # +c weights for the sq pass
    VWh[:, C * KD + 1] = 0.0
    vw_bytes = VWh.view(np.uint8)                        # (128, 1028)

    xs_all = (ALPHA * x * s[None, :]).astype(ml_dtypes.float8_e3m4)  # (B, 1024)

    in_maps = []
    for i in range(N_CORES):
        xs = xs_all[i * BS : (i + 1) * BS]               # (2048, 1024)
        xT = xs.T.reshape(C, 128, BS)                    # [c, p, b]
        xb = np.ascontiguousarray(
            xT[XORD].transpose(1, 0, 2).reshape(128, C * BS)
        ).view(np.uint8)
        XRb = np.concatenate([vw_bytes, xb], axis=1)     # (128, 17412)
        in_maps.append({"XR": XRb.view(ml_dtypes.float8_e3m4)})
    return in_maps


LAST_RESULTS = None  # stashed BassKernelResults (for test harness profiling)
TRACE = False


def kernel(x, vparam):
    global LAST_RESULTS
    nc = build_nc()
    in_maps = make_in_maps(x, vparam)
    res = run_bass_kernel_spmd(nc, in_maps, list(range(N_CORES)), trace=TRACE)
    LAST_RESULTS = res
    out = np.concatenate(
        [
            res.results[i]["OUT"].astype(np.float32).reshape(BS, 1)
            for i in range(N_CORES)
        ],
        axis=0,
    )
    return out.astype(np.float32)


# revision 17
# speedup vs baseline: 1.5264x; 1.0911x over previous
"""Trainium2 Bass kernel for nn_CrossProduct (factorization-machine cross term).

out_b = 0.5 * [ sum_k (x_b @ v_k)^2  -  sum_i w_i x_bi^2 ],  w_i = sum_k v_ik^2

Host-side rescaling removes all per-feature weights from the device:
  x~  = e3m4(4 * x * sqrt(w/2))     (shipped fp8e3m4, feature-on-partition)
  v~  = fp16(v / sqrt(w))           (bit-cast into the head of the X tensor)
  psA[k,b] = sum_i x~_bi v~_ik = 4 * (x v_k)/sqrt(2)     (mixed-dtype matmul)
  sq = (psA/4)^2 fp16, weighted by fp16(CBIAS) in the final reduce
  x2 = e4m3(CBIAS * x~^2) pair-interleaved; reduced on the PE via four
  DoubleRow ones(-1/16) passes (256-deep contraction each).  CBIAS is a
  non-pow2 scale that breaks e3m4-grid alignment so the e4m3 RTN cast of
  squares is mean-unbiased (plain cast has +0.75% convexity bias).
  out_b = (sum_k c*sq - (1/16) sum x2) / c  accumulated in psO row 0.

Schedule highlights:
  - Single dram tensor XR [128, 17412] e3m4, chunk order
    [vw | c0 | c1 | c6 | c7 | c2 | c3 | c4 | c5]:  sync HWDGE sends
    [vw,c0] then [c1,c6,c7]; gpsimd SWDGE sends [c2,c3] then [c4,c5].
    3-6KB descriptors keep both queues near the 16-DMA-engine aggregate.
  - 12 PE warm-up matmuls on a memset tile (no DMA dependency) hold the
    p-state clock up until real work arrives.
  - squares split ACT/DVE by (chunk, half) in arrival order; Pool does
    no tensor ops (they contend with DVE for SBUF ports).
  - finals in weights-major order: one ldweights per weight set.
"""

import math
from contextlib import ExitStack

import ml_dtypes
import numpy as np

import concourse.bass as bass
import concourse.bacc as bacc
import concourse.mybir as mybir
import concourse.tile as tile
from concourse.bass_utils import run_bass_kernel_spmd

F16 = mybir.dt.float16
F32 = mybir.dt.float32
F8E3 = mybir.dt.float8e3
F8E4 = mybir.dt.float8e4

N_CORES = 8
B, XD, KD = 16384, 1024, 64
BS = B // N_CORES   # 2048 batch rows per core
C = XD // 128       # 8 contraction chunks of 128
ALPHA = 4.0         # x pre-scale (exact pow2); term2 weight = 1/ALPHA^2
CBIAS = 1.17        # x2 pre-cast scale nulling the e4m3 RTN square bias
VWB = 1028          # vw bytes per partition at the head of XR
XRW = VWB + C * BS  # 17412 bytes per partition

# chunk order inside XR (after vw), chosen so the sync queue can deliver
# c0 early and both queues balance.
XORD = [0, 1, 6, 7, 2, 3, 4, 5]
XPOS = {c: i for i, c in enumerate(XORD)}  # chunk -> slot

SQ = mybir.ActivationFunctionType.Square
DR = mybir.MatmulPerfMode.DoubleRow


def _body(ctx, tc, OUT, XR):
    nc = tc.nc
    const = ctx.enter_context(tc.tile_pool(name="const", bufs=1))
    xpool = ctx.enter_context(tc.tile_pool(name="xp", bufs=1))
    x2pool = ctx.enter_context(tc.tile_pool(name="x2p", bufs=1))
    sqpool = ctx.enter_context(tc.tile_pool(name="sqp", bufs=1))
    opool = ctx.enter_context(tc.tile_pool(name="op", bufs=1))
    psa = ctx.enter_context(tc.tile_pool(name="psA", bufs=1, space="PSUM"))
    pso = ctx.enter_context(tc.tile_pool(name="psO", bufs=1, space="PSUM"))

    pa = psa.tile([64, BS], F32)
    po = pso.tile([1, BS], F32)

    # ---- PE warm-up (alternating psO bank regions so they pipeline).
    warm = const.tile([128, 512], F16)
    nc.vector.memset(warm, 0.0)
    for i in range(8):
        cols = slice(512 * (i % 2), 512 * (i % 2) + 512)
        nc.tensor.matmul(
            po[0:1, cols], warm[:, 0:1], warm[:],
            start=True, stop=True, tile_position=(0, 0),
        )

    # ---- XR input.
    xr = xpool.tile([128, XRW], F8E3)

    def xsl(lo, hi):  # slot-range bounds (bytes, after vw head)
        return slice(VWB + lo * BS, VWB + hi * BS)

    # Both X streams ride the two HWDGE queues: SWDGE (gpsimd) completion
    # semaphores lag ~2.2us behind the last packet, HWDGE ones don't.
    nc.sync.dma_start(xr[:, 0 : VWB + BS], XR[:, 0 : VWB + BS])      # vw,c0
    nc.scalar.dma_start(xr[:, xsl(4, 6)], XR[:, xsl(4, 6)])          # c2,c3
    nc.sync.dma_start(xr[:, xsl(1, 4)], XR[:, xsl(1, 4)])            # c1,c6,c7
    nc.scalar.dma_start(xr[:, xsl(6, 8)], XR[:, xsl(6, 8)])          # c4,c5

    # vw view: [128, 514] fp16 = [c*64+k -> v~ ; col 512 = +CBIAS(fp16)]
    vw = xr[:, 0:VWB].bitcast(F16)

    def xch(c):
        i = XPOS[c]
        return xr[:, VWB + i * BS : VWB + (i + 1) * BS]

    # ---- pa matmuls in arrival order: fp16 weights x e3m4 moving.
    def pa_mm(c, first, last):
        for q in range(4):
            nc.tensor.matmul(
                pa[:, q * 512 : (q + 1) * 512],
                vw[:, c * KD : (c + 1) * KD],
                xch(c)[:, q * 512 : (q + 1) * 512],
                start=first,
                stop=last,
                tile_position=(0, 0),
            )

    order = [0, 2, 3, 1, 6, 7, 4, 5]
    for i, c in enumerate(order):
        pa_mm(c, i == 0, i == len(order) - 1)

    # ---- squares: x2 = e4m3(CBIAS * x~^2), pair-interleaved for DR.
    # x2[:, cc, j, :] holds chunk (2cc+j)^2.
    x2 = x2pool.tile([128, C // 2, 2, BS], F8E4)
    H = BS // 2
    CSQ = math.sqrt(CBIAS)

    def x2ap(c, h):
        return x2[:, c // 2, c % 2, h * H : (h + 1) * H]

    def sq_act(c, h):
        nc.scalar.activation(x2ap(c, h), xch(c)[:, h * H : (h + 1) * H], SQ,
                             scale=CSQ)

    def sq_dve(c, h):
        nc.vector.scalar_tensor_tensor(
            x2ap(c, h), xch(c)[:, h * H : (h + 1) * H], CBIAS,
            xch(c)[:, h * H : (h + 1) * H],
            mybir.AluOpType.mult, mybir.AluOpType.mult)

    # arrival: c0 ~11us; c2,c3 ~13.6; c1,c6,c7 ~15; c4,c5 ~16.3
    for c, h in [(0, 0), (0, 1), (2, 0), (2, 1), (6, 0), (6, 1), (4, 0), (4, 1)]:
        sq_act(c, h)
    for c, h in [(3, 0), (3, 1), (1, 0), (1, 1), (7, 0), (7, 1), (5, 0), (5, 1)]:
        sq_dve(c, h)

    # ---- sq = (psA/4)^2 fp16 on ACT (DVE cannot read two PSUM operands).
    sqs = sqpool.tile([64, BS], F16)
    nc.scalar.activation(sqs[:], pa[:], SQ, scale=1.0 / ALPHA)

    # ---- finals into psO row 0: weights-major order (one ldweights per
    # weight set): 4 DR ones(-1/16) passes over x2, then the +c sq pass.
    wneg8 = const.tile([128, 2, 16], F8E4)
    nc.vector.memset(wneg8, -1.0 / (ALPHA * ALPHA))
    onesc = vw[0:64, 512:513]
    for cc in range(4):
        for q in range(4):
            cols = slice(q * 512, (q + 1) * 512)
            nc.tensor.matmul(
                po[0:1, cols],
                wneg8[:, :, 0:1],
                x2[:, cc, :, cols],
                start=(cc == 0),
                stop=False,
                perf_mode=DR,
                tile_position=(0, 0),
            )
    for q in range(4):
        cols = slice(q * 512, (q + 1) * 512)
        nc.tensor.matmul(
            po[0:1, cols],
            onesc,
            sqs[:, cols],
            start=False,
            stop=True,
            tile_position=(0, 0),
        )

    outs = opool.tile([1, BS], F16)
    nc.scalar.mul(outs[0:1, 0:H], po[0:1, 0:H], 1.0 / CBIAS)
    nc.vector.tensor_scalar_mul(outs[0:1, H:BS], po[0:1, H:BS], 1.0 / CBIAS)
    nc.sync.dma_start(OUT, outs[0:1, :])


_NC_CACHE = None


def build_nc():
    global _NC_CACHE
    if _NC_CACHE is not None:
        return _NC_CACHE
    nc = bacc.Bacc("TRN2", target_bir_lowering=False, debug=False)
    XR = nc.dram_tensor("XR", [128, XRW], F8E3, kind="ExternalInput").ap()
    OUT = nc.dram_tensor("OUT", [1, BS], F16, kind="ExternalOutput").ap()
    with tile.TileContext(nc) as tc:
        with ExitStack() as ctx:
            _body(ctx, tc, OUT, XR)
    nc.compile()
    _NC_CACHE = nc
    return nc


def make_in_maps(x, vparam):
    x = np.ascontiguousarray(x, dtype=np.float32)
    v = np.ascontiguousarray(vparam, dtype=np.float32)

    w = (v.astype(np.float64) ** 2).sum(axis=1)          # (1024,)
    w = np.maximum(w, 1e-12)
    s = np.sqrt(w / 2.0)
    vn = (v / np.sqrt(w)[:, None]).astype(np.float32)    # (1024, 64)

    VWh = np.empty((128, VWB // 2), dtype=np.float16)
    # VW[p, c*64+k] = vn[c*128+p, k]
    VWh[:, 0 : C * KD] = (
        vn.reshape(C, 128, KD).transpose(1, 0, 2).reshape(128, C * KD)
    )
    VWh[:, C * KD] = np.float16(CBIAS)   # +c weights for the sq pass
    VWh[:, C * KD + 1] = 0.0
    vw_bytes = VWh.view(np.uint8)                        # (128, 1028)

    xs_all = (ALPHA * x * s[None, :]).astype(ml_dtypes.float8_e3m4)  # (B, 1024)

    in_maps = []
    for i in range(N_CORES):
        xs = xs_all[i * BS : (i + 1) * BS]               # (2048, 1024)
        xT = xs.T.reshape(C, 128, BS)                    # [c, p, b]
        xb = np.ascontiguousarray(
            xT[XORD].transpose(1, 0, 2).reshape(128, C * BS)
        ).view(np.uint8)
        XRb = np.concatenate([vw_bytes, xb], axis=1)     # (128, 17412)
        in_maps.append({"XR": XRb.view(ml_dtypes.float8_e3m4)})
    return in_maps


LAST_RESULTS = None  # stashed BassKernelResults (for test harness profiling)
TRACE = False


def kernel(x, vparam):
    global LAST_RESULTS
    nc = build_nc()
    in_maps = make_in_maps(x, vparam)
    res = run_bass_kernel_spmd(nc, in_maps, list(range(N_CORES)), trace=TRACE)
    LAST_RESULTS = res
    out = np.concatenate(
        [
            res.results[i]["OUT"].astype(np.float32).reshape(BS, 1)
            for i in range(N_CORES)
        ],
        axis=0,
    )
    return out.astype(np.float32)
